# revision 2
# baseline (speedup 1.0000x reference)
"""Chamfer loss (squared-distance NN, both directions) on 8 Trainium2 cores.

Strategy
--------
Data-parallel over the batch: core b handles point clouds x[b], y[b]
(N=4096 points, C=3).  On each core the 4096x4096 *negated* squared
distance matrix is produced stripe-by-stripe ([128, 2048] PSUM groups)
by a single augmented matmul:

    psum[i, j] = 2*x_i.y_j - |x_i|^2 - |y_j|^2   (= -pdist[i, j])

The augmentation packs the cross term and both norm terms into a K=16
contraction where every fp32 value is represented as a bf16 hi+lo pair
(relative error ~2^-16), so the matmul runs at full bf16 PE speed while
keeping near-fp32 distance accuracy.

Engine split per [128, 2048] PSUM group (the drain is the bottleneck,
not the PE):
  * Scalar (ACT) converts the group fp32->bf16 into SBUF (1 el/cyc).
  * Vector folds the group into the y-side accumulator with one
    2x-mode tensor_tensor max (2 el/cyc).
  * Vector then handles the whole x-side of a stripe with ONE 4x-mode
    tensor_scalar (op0=max with -inf => identity, accum_out=max-reduce
    along free axis): 4 el/cyc, writing the row minima directly.

Each core returns 4096 row maxima ([128, 32] fp32) and the y-side
accumulator ([128, 4096] bf16); the host negates, finishes the y-side
partition max, averages, and scales
(loss = 0.005 * (mean min_j + mean min_i)).
"""

import numpy as np
import ml_dtypes

import concourse.bass as bass
import concourse.bass_isa as bass_isa
import concourse.mybir as mybir
import concourse.tile as tile
from concourse.bass_utils import run_bass_kernel_spmd

B = 8          # batches == cores
N = 4096       # points per cloud
P = 128        # row-tile size (PSUM partitions)
NB = 512       # matmul free-dim (one PSUM bank of fp32)
GB = 2048      # drain group: 4 banks handled by one ACT/DVE op
MT = N // P    # 32 row stripes
NG = N // GB   # 2 column groups per stripe
K = 16         # augmented contraction length

BF16 = ml_dtypes.bfloat16

NEG_INF_BF16 = float(ml_dtypes.finfo(BF16).min)


def _build_program() -> bass.Bass:
    nc = bass.Bass("TRN2", target_bir_lowering=False, debug=False)

    xa = nc.dram_tensor("xa", [K, N], mybir.dt.bfloat16, kind="ExternalInput").ap()
    ya = nc.dram_tensor("ya", [K, N], mybir.dt.bfloat16, kind="ExternalInput").ap()
    xmax_d = nc.dram_tensor("xmax", [P, MT], mybir.dt.float32, kind="ExternalOutput").ap()
    ymax_d = nc.dram_tensor(
        "ymax", [P, N], mybir.dt.bfloat16, kind="ExternalOutput"
    ).ap()

    with tile.TileContext(nc) as tc:
        with (
            tc.tile_pool(name="inp", bufs=1) as inp_pool,
            tc.tile_pool(name="psum", bufs=2, space="PSUM") as psum_pool,
            tc.tile_pool(name="cast", bufs=3) as cast_pool,
            tc.tile_pool(name="accy", bufs=1) as accy_pool,
            tc.tile_pool(name="res", bufs=1) as res_pool,
        ):
            xa_sb = inp_pool.tile([K, N], mybir.dt.bfloat16)
            ya_sb = inp_pool.tile([K, N], mybir.dt.bfloat16)
            nc.sync.dma_start(out=xa_sb, in_=xa)
            nc.sync.dma_start(out=ya_sb, in_=ya)

            acc_y = accy_pool.tile([P, N], mybir.dt.bfloat16)
            xres = res_pool.tile([P, MT], mybir.dt.float32)
            for m in range(MT):
                ct = cast_pool.tile([P, N], mybir.dt.bfloat16, tag="ct")
                for g in range(NG):
                    pt = psum_pool.tile([P, GB], mybir.dt.float32, tag="pt")
                    for i in range(GB // NB):
                        j0 = g * GB + i * NB
                        nc.tensor.matmul(
                            out=pt[:, i * NB : (i + 1) * NB],
                            lhsT=xa_sb[:, m * P : (m + 1) * P],
                            rhs=ya_sb[:, j0 : j0 + NB],
                            start=True,
                            stop=True,
                        )
                    cslice = ct[:, g * GB : (g + 1) * GB]
                    nc.scalar.copy(out=cslice, in_=pt)
                    if m == 0:
                        nc.vector.tensor_copy(
                            out=acc_y[:, g * GB : (g + 1) * GB], in_=cslice
                        )
                    else:
                        nc.vector.tensor_max(
                            out=acc_y[:, g * GB : (g + 1) * GB],
                            in0=acc_y[:, g * GB : (g + 1) * GB],
                            in1=cslice,
                        )
                # Whole-stripe x-side: out = max(ct, -inf) = ct (in-place,
                # 4x mode), accum = max-reduce along the free axis.
                nc.vector.tensor_scalar(
                    out=ct,
                    in0=ct,
                    scalar1=NEG_INF_BF16,
                    scalar2=None,
                    op0=mybir.AluOpType.max,
                    op1=mybir.AluOpType.max,
                    accum_out=xres[:, m : m + 1],
                )

            nc.sync.dma_start(out=xmax_d, in_=xres)
            nc.sync.dma_start(out=ymax_d, in_=acc_y)

    _split_excess_waits(nc)
    return nc


def _split_excess_waits(nc: bass.Bass) -> None:
    """Walrus codegen fits exactly one sync wait per instruction struct.

    For any scheduled instruction carrying more, move all but the last wait
    onto same-engine NoOps inserted immediately before it — the engine's
    sequencer then processes the same waits in the same order.
    """
    k = 0
    for f in nc.m.functions:
        for b in f.blocks:
            out = []
            for inst in b.instructions:
                si = inst.sync_info
                if si is not None and si.on_wait and len(si.on_wait) > 1:
                    waits = list(si.on_wait)
                    for w in waits[:-1]:
                        nop = mybir.InstNoOp(
                            name=f"ws-{k}", text_hint="wait_split"
                        )
                        k += 1
                        nop.engine = inst.engine
                        nop.sync_info = mybir.SyncInfo(on_wait=[w], on_update=[])
                        out.append(nop)
                    inst.sync_info = mybir.SyncInfo(
                        on_wait=[waits[-1]], on_update=list(si.on_update or [])
                    )
                out.append(inst)
            b.instructions = out


def _split_bf16(a: np.ndarray):
    """hi + lo bf16 pair with hi+lo ~= a (a is float64)."""
    hi = a.astype(BF16)
    lo = (a - hi.astype(np.float64)).astype(BF16)
    return hi, lo


def _prep_core(xb: np.ndarray, yb: np.ndarray):
    """Build the [K, N] augmented bf16 operands for one batch.

    Row pairing (XA[k] multiplies YA[k], summed over k):
      0-2 : xh * yh2   3-5 : xh * yl2   6-8 : xl * yh2   9-11: xl * yl2
      12  : mxh * 1    13  : mxl * 1    14  : 1 * myh    15  : 1 * myl
    where (xh+xl) ~= x, (yh2+yl2) ~= 2*y, (mxh+mxl) ~= -|x|^2,
    (myh+myl) ~= -|y|^2.
    """
    xt = xb.T.astype(np.float64)  # [3, N]
    yt = yb.T.astype(np.float64)
    xh, xl = _split_bf16(xt)
    yh, yl = _split_bf16(2.0 * yt)
    mxh, mxl = _split_bf16(-np.sum(xt * xt, axis=0, keepdims=True))
    myh, myl = _split_bf16(-np.sum(yt * yt, axis=0, keepdims=True))
    ones = np.ones((1, N), dtype=BF16)

    XA = np.concatenate([xh, xh, xl, xl, mxh, mxl, ones, ones], axis=0)
    YA = np.concatenate([yh, yl, yh, yl, ones, ones, myh, myl], axis=0)
    assert XA.shape == (K, N) and YA.shape == (K, N)
    return np.ascontiguousarray(XA), np.ascontiguousarray(YA)


_NC_CACHE: list = []


def _get_program() -> bass.Bass:
    if not _NC_CACHE:
        _NC_CACHE.append(_build_program())
    return _NC_CACHE[0]


def _run(x: np.ndarray, y: np.ndarray, **spmd_kwargs):
    """Run the SPMD kernel; returns (loss_f32, BassKernelResults)."""
    x = np.asarray(x, dtype=np.float32)
    y = np.asarray(y, dtype=np.float32)
    assert x.shape == (B, N, 3) and y.shape == (B, N, 3), (x.shape, y.shape)

    nc = _get_program()
    in_maps = []
    for b in range(B):
        XA, YA = _prep_core(x[b], y[b])
        in_maps.append({"xa": XA, "ya": YA})

    res = run_bass_kernel_spmd(nc, in_maps, core_ids=list(range(B)), **spmd_kwargs)

    sx = 0.0
    sy = 0.0
    for r in res.results:
        sx += -r["xmax"].astype(np.float64).sum()
        ymax = np.asarray(r["ymax"]).astype(np.float32)
        sy += -ymax.max(axis=0).astype(np.float64).sum()
    loss = 0.005 * (sx / (B * N) + sy / (B * N))
    return np.float32(loss), res


def kernel(x: np.ndarray, y: np.ndarray) -> np.ndarray:
    loss, _ = _run(x, y)
    return loss


# revision 3
# speedup vs baseline: 4.5678x; 4.5678x over previous
"""Chamfer loss (squared-distance NN, both directions) on 8 Trainium2 cores.

Strategy
--------
Data-parallel over the batch: core b handles point clouds x[b], y[b]
(N=4096 points, C=3).  On each core the 4096x4096 *negated* squared
distance matrix is produced stripe-by-stripe ([128, 2048] PSUM groups)
by a single augmented matmul:

    psum[i, j] = 2*x_i.y_j - |x_i|^2 - |y_j|^2   (= -pdist[i, j])

The augmentation packs the cross term and both norm terms into a K=16
contraction where every fp32 value is represented as a bf16 hi+lo pair
(relative error ~2^-16), so the matmul runs at full bf16 PE speed while
keeping near-fp32 distance accuracy.

Engine split per [128, 2048] PSUM group (the drain is the bottleneck,
not the PE):
  * Scalar (ACT) converts the group fp32->bf16 into SBUF (1 el/cyc).
  * Vector folds the group into the y-side accumulator with one
    2x-mode tensor_tensor max (2 el/cyc).
  * Vector then handles the whole x-side of a stripe with ONE 4x-mode
    tensor_scalar (op0=max with -inf => identity, accum_out=max-reduce
    along free axis): 4 el/cyc, writing the row minima directly.

Each core returns 4096 row maxima ([128, 32] fp32) and the y-side
accumulator ([128, 4096] bf16); the host negates, finishes the y-side
partition max, averages, and scales
(loss = 0.005 * (mean min_j + mean min_i)).
"""

import numpy as np
import ml_dtypes

import concourse.bass as bass
import concourse.bass_isa as bass_isa
import concourse.mybir as mybir
import concourse.tile as tile
from concourse.bass_utils import run_bass_kernel_spmd

B = 8          # batches == cores
N = 4096       # points per cloud
P = 128        # row-tile size (PSUM partitions)
NB = 512       # matmul free-dim (one PSUM bank of fp32)
GB = 2048      # drain group: 4 banks handled by one ACT/DVE op
MT = N // P    # 32 row stripes
NG = N // GB   # 2 column groups per stripe
K = 16         # augmented contraction length

BF16 = ml_dtypes.bfloat16

NEG_INF_BF16 = float(ml_dtypes.finfo(BF16).min)


def _build_program() -> bass.Bass:
    nc = bass.Bass("TRN2", target_bir_lowering=False, debug=False)

    xa = nc.dram_tensor("xa", [K, N], mybir.dt.bfloat16, kind="ExternalInput").ap()
    ya = nc.dram_tensor("ya", [K, N], mybir.dt.bfloat16, kind="ExternalInput").ap()
    xmax_d = nc.dram_tensor("xmax", [P, MT], mybir.dt.bfloat16, kind="ExternalOutput").ap()
    ymax_d = nc.dram_tensor(
        "ymax", [P, N], mybir.dt.bfloat16, kind="ExternalOutput"
    ).ap()

    with tile.TileContext(nc) as tc:
        with (
            tc.tile_pool(name="inp", bufs=1) as inp_pool,
            tc.tile_pool(name="psum", bufs=2, space="PSUM") as psum_pool,
            tc.tile_pool(name="cast", bufs=3) as cast_pool,
            tc.tile_pool(name="accy", bufs=1) as accy_pool,
            tc.tile_pool(name="res", bufs=1) as res_pool,
        ):
            xa_sb = inp_pool.tile([K, N], mybir.dt.bfloat16)
            ya_sb = inp_pool.tile([K, N], mybir.dt.bfloat16)
            nc.sync.dma_start(out=xa_sb, in_=xa)
            nc.sync.dma_start(out=ya_sb, in_=ya)

            acc_y = accy_pool.tile([P, N], mybir.dt.bfloat16)
            xres = res_pool.tile([P, MT], mybir.dt.bfloat16)
            for m in range(MT):
                ct = cast_pool.tile([P, N], mybir.dt.bfloat16, tag="ct")
                for g in range(NG):
                    pt = psum_pool.tile([P, GB], mybir.dt.float32, tag="pt")
                    for i in range(GB // NB):
                        j0 = g * GB + i * NB
                        nc.tensor.matmul(
                            out=pt[:, i * NB : (i + 1) * NB],
                            lhsT=xa_sb[:, m * P : (m + 1) * P],
                            rhs=ya_sb[:, j0 : j0 + NB],
                            start=True,
                            stop=True,
                        )
                    cslice = ct[:, g * GB : (g + 1) * GB]
                    nc.scalar.copy(out=cslice, in_=pt)
                    if m == 0:
                        nc.vector.tensor_copy(
                            out=acc_y[:, g * GB : (g + 1) * GB], in_=cslice
                        )
                    else:
                        nc.vector.tensor_max(
                            out=acc_y[:, g * GB : (g + 1) * GB],
                            in0=acc_y[:, g * GB : (g + 1) * GB],
                            in1=cslice,
                        )
                # Whole-stripe x-side: out = max(ct, -inf) = ct (in-place,
                # 4x mode), accum = max-reduce along the free axis.
                nc.vector.tensor_scalar(
                    out=ct,
                    in0=ct,
                    scalar1=NEG_INF_BF16,
                    scalar2=None,
                    op0=mybir.AluOpType.max,
                    op1=mybir.AluOpType.max,
                    accum_out=xres[:, m : m + 1],
                )

            nc.sync.dma_start(out=xmax_d, in_=xres)
            nc.sync.dma_start(out=ymax_d, in_=acc_y)

    _split_excess_waits(nc)
    return nc


def _split_excess_waits(nc: bass.Bass) -> None:
    """Walrus codegen fits exactly one sync wait per instruction struct.

    For any scheduled instruction carrying more, move all but the last wait
    onto same-engine NoOps inserted immediately before it — the engine's
    sequencer then processes the same waits in the same order.
    """
    k = 0
    for f in nc.m.functions:
        for b in f.blocks:
            out = []
            for inst in b.instructions:
                si = inst.sync_info
                if si is not None and si.on_wait and len(si.on_wait) > 1:
                    waits = list(si.on_wait)
                    for w in waits[:-1]:
                        nop = mybir.InstNoOp(
                            name=f"ws-{k}", text_hint="wait_split"
                        )
                        k += 1
                        nop.engine = inst.engine
                        nop.sync_info = mybir.SyncInfo(on_wait=[w], on_update=[])
                        out.append(nop)
                    inst.sync_info = mybir.SyncInfo(
                        on_wait=[waits[-1]], on_update=list(si.on_update or [])
                    )
                out.append(inst)
            b.instructions = out


def _split_bf16(a: np.ndarray):
    """hi + lo bf16 pair with hi+lo ~= a (a is float64)."""
    hi = a.astype(BF16)
    lo = (a - hi.astype(np.float64)).astype(BF16)
    return hi, lo


def _prep_core(xb: np.ndarray, yb: np.ndarray):
    """Build the [K, N] augmented bf16 operands for one batch.

    Row pairing (XA[k] multiplies YA[k], summed over k):
      0-2 : xh * yh2   3-5 : xh * yl2   6-8 : xl * yh2   9-11: xl * yl2
      12  : mxh * 1    13  : mxl * 1    14  : 1 * myh    15  : 1 * myl
    where (xh+xl) ~= x, (yh2+yl2) ~= 2*y, (mxh+mxl) ~= -|x|^2,
    (myh+myl) ~= -|y|^2.
    """
    xt = xb.T.astype(np.float64)  # [3, N]
    yt = yb.T.astype(np.float64)
    xh, xl = _split_bf16(xt)
    yh, yl = _split_bf16(2.0 * yt)
    mxh, mxl = _split_bf16(-np.sum(xt * xt, axis=0, keepdims=True))
    myh, myl = _split_bf16(-np.sum(yt * yt, axis=0, keepdims=True))
    ones = np.ones((1, N), dtype=BF16)

    XA = np.concatenate([xh, xh, xl, xl, mxh, mxl, ones, ones], axis=0)
    YA = np.concatenate([yh, yl, yh, yl, ones, ones, myh, myl], axis=0)
    assert XA.shape == (K, N) and YA.shape == (K, N)
    return np.ascontiguousarray(XA), np.ascontiguousarray(YA)


_NC_CACHE: list = []


def _get_program() -> bass.Bass:
    if not _NC_CACHE:
        _NC_CACHE.append(_build_program())
    return _NC_CACHE[0]


def _run(x: np.ndarray, y: np.ndarray, **spmd_kwargs):
    """Run the SPMD kernel; returns (loss_f32, BassKernelResults)."""
    x = np.asarray(x, dtype=np.float32)
    y = np.asarray(y, dtype=np.float32)
    assert x.shape == (B, N, 3) and y.shape == (B, N, 3), (x.shape, y.shape)

    nc = _get_program()
    in_maps = []
    for b in range(B):
        XA, YA = _prep_core(x[b], y[b])
        in_maps.append({"xa": XA, "ya": YA})

    res = run_bass_kernel_spmd(nc, in_maps, core_ids=list(range(B)), **spmd_kwargs)

    sx = 0.0
    sy = 0.0
    for r in res.results:
        sx += -r["xmax"].astype(np.float64).sum()
        ymax = np.asarray(r["ymax"]).astype(np.float32)
        sy += -ymax.max(axis=0).astype(np.float64).sum()
    loss = 0.005 * (sx / (B * N) + sy / (B * N))
    return np.float32(loss), res


def kernel(x: np.ndarray, y: np.ndarray) -> np.ndarray:
    loss, _ = _run(x, y)
    return loss
